# revision 21
# baseline (speedup 1.0000x reference)
"""FAGCN forward on 8 TRN2 NeuronCores (Bass/Tile) — ELL edition.

Sharding: row-partition of nodes, 8 ways. The dense input projection
(h0 = relu(x @ t1^T + b)) is computed replicated into a DRAM table of
512B rows [h bf16 x128 | b f32 | pad]. Per layer the edge phase runs
TWO independent ELL structures (edges split by col half so gather
indices fit int16): own rows are degree-sorted into windows of 128
(partition = target row), edge slots are gathered per window with
dma_gather, the gate is tanh(b_col + bias(a_row+gb)) on ScalarE (a_row
is a per-partition constant in this layout), messages are formed with
ONE broadcast tensor_tensor per chunk, and the scatter-add is an
identity-stationary matmul accumulating in PSUM. Window results are
dma_scatter_add-ed (CCE f32) into a DRAM accumulator; a combine pass
adds eps*h0 and builds the next table / the head. Head log_softmax is
deferred and batched to avoid activation-table thrash.
"""

import os
import sys
import numpy as np

sys.path.insert(0, "/opt/trn_rl_repo")

import concourse.bass as bass
import concourse.bacc as bacc
import concourse.mybir as mybir
import concourse.tile as tile
from concourse import library_config

F32 = mybir.dt.float32
BF16 = mybir.dt.bfloat16
I16 = mybir.dt.int16

N_NODES = 50000
IN_CH = 256
HIDDEN = 128
OUT_CH = 64
EPS = 0.3
NCORES = 8
MAXSLOT = 32      # slots per gather chunk (32*128 = 4096 idxs)
GW = 4            # windows per scatter_add batch


def _install_profile_hook():
    import types
    name = "antenv.axon_hooks"
    if name in sys.modules:
        return
    try:
        import trn_agent_boot.trn_boot as tb
        hook = tb._ntff_profile_via_ctypes("/opt/axon/libaxon_pjrt.so")
    except Exception:
        hook = None
    mod = types.ModuleType(name)
    mod._hook = hook
    mod.get_axon_ntff_profile_hook = lambda: mod._hook
    mod.set_axon_ntff_profile_hook = lambda h: setattr(mod, "_hook", h)
    sys.modules[name] = mod


def _wrap16(arr):
    n = arr.shape[0]
    assert n % 16 == 0
    return np.tile(np.ascontiguousarray(arr.reshape(n // 16, 16).T), (8, 1))


# ======================================================================
# Host preprocessing
# ======================================================================

def preprocess(edge_index, n_nodes, ncores, lo_split):
    row = np.asarray(edge_index[0], np.int64)
    col = np.asarray(edge_index[1], np.int64)
    r_per = n_nodes // ncores
    nwin = (r_per + 127) // 128
    padr = nwin * 128
    assert lo_split % r_per == 0
    locnt = lo_split // r_per
    hibase = locnt * padr

    deg = np.bincount(row, minlength=n_nodes).astype(np.float64)
    dinv = np.where(deg > 0, 1.0 / np.sqrt(np.maximum(deg, 1.0)), 0.0)
    esc_all = (dinv[row] * dinv[col]).astype(np.float32)
    core = row // r_per
    lrow = row - core * r_per
    is_hi = (col >= lo_split).astype(np.int64)

    # pass 1: per (core, S) degree profile; window depths shared across cores
    nd_sorted = np.zeros((ncores, 2, padr), np.int64)
    perms = {}
    for c in range(ncores):
        own = core == c
        for S in (0, 1):
            m = own & (is_hi == S)
            nd = np.bincount(lrow[m], minlength=r_per)
            pf = np.argsort(-nd, kind="stable")
            perms[(c, S)] = pf
            nd_sorted[c, S, :r_per] = nd[pf]
    Dw = np.maximum(nd_sorted.max(axis=0).reshape(2, nwin, 128).max(axis=2), 1)
    slot_base = np.concatenate(
        [np.zeros((2, 1), np.int64), np.cumsum(Dw, axis=1)[:, :-1]], axis=1)
    tot_slots = Dw.sum(axis=1).astype(np.int64)

    pieces = {S: [] for S in (0, 1)}
    for S in (0, 1):
        for w in range(nwin):
            off = 0
            while off < Dw[S, w]:
                ns = int(min(MAXSLOT, Dw[S, w] - off))
                pieces[S].append(
                    (int(slot_base[S, w] + off), ns, w, off == 0, off + ns == Dw[S, w]))
                off += ns

    percore = []
    for c in range(ncores):
        own = core == c
        sdata = []
        for S in (0, 1):
            m = own & (is_hi == S)
            lr = lrow[m]
            cl = col[m]
            es = esc_all[m]
            o = np.lexsort((cl, lr))
            lr, cl, es = lr[o], cl[o], es[o]
            pf = perms[(c, S)]
            pos_of = np.empty(r_per, np.int64)
            pos_of[pf] = np.arange(r_per)
            first = np.searchsorted(lr, np.arange(r_per))
            seq = np.arange(lr.shape[0]) - first[lr]
            pos = pos_of[lr]
            token = (slot_base[S, pos // 128] + seq) * 128 + (pos % 128)
            ntok = int(tot_slots[S]) * 128
            tloc0 = np.where(S == 1, cl - lo_split, cl)
            q = cl // r_per
            agorow = q * padr + (cl - q * r_per)
            tloc1 = agorow - (hibase if S == 1 else 0)
            assert len(tloc0) == 0 or (tloc0.min() >= 0 and tloc0.max() < 32768)
            assert len(tloc1) == 0 or (tloc1.min() >= 0 and tloc1.max() < 32768)
            idx0 = np.zeros(ntok, np.int16)
            idx1 = np.zeros(ntok, np.int16)
            esct = np.zeros(ntok, np.float32)
            idx0[token] = tloc0.astype(np.int16)
            idx1[token] = tloc1.astype(np.int16)
            esct[token] = es
            perm_pad = np.concatenate([pf, np.arange(r_per, padr)])
            sdata.append({
                "idx0": _wrap16(idx0), "idx1": _wrap16(idx1),
                "es": np.ascontiguousarray(esct.reshape(-1, 128).T),
                "aidx": _wrap16(np.where(perm_pad < r_per, perm_pad, 0).astype(np.int16)),
                "sidx": _wrap16(perm_pad.astype(np.int16)),
                "perm": pf,
            })
        percore.append(sdata)

    return {
        "Dw": Dw, "slot_base": slot_base, "tot_slots": tot_slots,
        "pieces": pieces, "percore": percore, "nwin": nwin, "padr": padr,
        "locnt": locnt, "hibase": hibase,
    }




def _mr(nc, scr, in0, in1, accum_out):
    """accum_out = sum(in0 * in1) along free dim (TTR is broken on HW)."""
    nc.vector.tensor_tensor(out=scr, in0=in0, in1=in1, op=mybir.AluOpType.mult)
    nc.vector.reduce_sum(out=accum_out, in_=scr, axis=mybir.AxisListType.X)


# ======================================================================
# Kernel builder
# ======================================================================

def build_kernel(meta, n_nodes, in_ch, hidden, out_ch, eps, lo_split, ncores):
    nwin = meta["nwin"]
    padr = meta["padr"]
    hibase = meta["hibase"]
    tot = [int(meta["tot_slots"][0]), int(meta["tot_slots"][1])]
    pieces = meta["pieces"]
    r_per = n_nodes // ncores
    nchunk = (n_nodes + 127) // 128
    last_win_rows = r_per - 128 * (nwin - 1)
    kt = in_ch // 128
    hh = hidden // 2  # f32 slots holding bf16 h
    knq = int(os.environ.get("KNQ", "4"))
    ngroups = (nwin + GW - 1) // GW

    nc = bacc.Bacc("TRN2", target_bir_lowering=False, debug=False,
                   num_devices=ncores, num_swdge_queues=knq)

    # ---- I/O ----
    xt = nc.dram_tensor("xt", [nchunk, in_ch, 128], BF16, kind="ExternalInput")
    xball = nc.dram_tensor("xball", [nchunk, 128], BF16, kind="ExternalInput")
    # own rows: natural order + one permuted copy per structure
    xto = nc.dram_tensor("xto", [3, nwin, in_ch, 128], BF16, kind="ExternalInput")
    xtob = nc.dram_tensor("xtob", [3, nwin, 128], BF16, kind="ExternalInput")
    t1wt = nc.dram_tensor("t1wt", [in_ch + 1, hidden], BF16, kind="ExternalInput")
    gwrep = nc.dram_tensor("gwrep", [4, 128, hidden], BF16, kind="ExternalInput")
    gbrep = nc.dram_tensor("gbrep", [128, 2], F32, kind="ExternalInput")
    t2wt = nc.dram_tensor("t2wt", [hidden, out_ch], F32, kind="ExternalInput")
    t2b = nc.dram_tensor("t2b", [1, out_ch], F32, kind="ExternalInput")
    identf_in = nc.dram_tensor("identf", [128, 128], F32, kind="ExternalInput")
    identb_in = nc.dram_tensor("identb", [128, 128], BF16, kind="ExternalInput")
    ones_in = nc.dram_tensor("ones", [1, 128], F32, kind="ExternalInput")
    idx_in = {}
    es_in = {}
    ax_in = {}
    sx_in = {}
    for S in (0, 1):
        idx_in[(0, S)] = nc.dram_tensor(f"idx0{S}", [128, tot[S] * 8], I16, kind="ExternalInput")
        idx_in[(1, S)] = nc.dram_tensor(f"idx1{S}", [128, tot[S] * 8], I16, kind="ExternalInput")
        es_in[S] = nc.dram_tensor(f"es{S}", [128, tot[S]], F32, kind="ExternalInput")
        ax_in[S] = nc.dram_tensor(f"ax{S}", [128, padr // 16], I16, kind="ExternalInput")
        sx_in[S] = nc.dram_tensor(f"sx{S}", [128, padr // 16], I16, kind="ExternalInput")
    out = nc.dram_tensor("out", [r_per, out_ch], F32, kind="ExternalOutput")

    ext0 = nc.dram_tensor("ext0", [nchunk * 128, 128], F32)
    agg = nc.dram_tensor("agg", [padr, 128], F32)
    agi = nc.dram_tensor("agi", [padr, 128], F32)
    ago = nc.dram_tensor("ago", [padr * ncores, 128], F32)

    with tile.TileContext(nc) as tc:
        nc.gpsimd.load_library(library_config.mlp)
        with tc.tile_pool(name="consts", bufs=1) as cp:
            t1wt_sb = cp.tile([128, kt, hidden], BF16, tag="t1wt")
            nc.sync.dma_start(t1wt_sb[:], bass.AP(t1wt, 0, [[hidden, 128], [128 * hidden, kt], [1, hidden]]))
            t1b_sb = cp.tile([1, hidden], BF16, tag="t1b")
            nc.sync.dma_start(t1b_sb[:], t1wt.ap()[in_ch:in_ch + 1, :])
            gw_sb = cp.tile([128, 4, hidden], BF16, tag="gw")
            nc.sync.dma_start(gw_sb[:], bass.AP(gwrep, 0, [[hidden, 128], [128 * hidden, 4], [1, hidden]]))
            gb_sb = cp.tile([128, 2], F32, tag="gb")
            nc.sync.dma_start(gb_sb[:], gbrep.ap())
            t2wt_sb = cp.tile([128, out_ch], F32, tag="t2wt")
            nc.sync.dma_start(t2wt_sb[:], t2wt.ap())
            t2b_sb = cp.tile([1, out_ch], F32, tag="t2b")
            nc.sync.dma_start(t2b_sb[:], t2b.ap())
            identf_sb = cp.tile([128, 128], F32, tag="identf")
            nc.sync.dma_start(identf_sb[:], identf_in.ap())
            identb_sb = cp.tile([128, 128], BF16, tag="identb")
            nc.sync.dma_start(identb_sb[:], identb_in.ap())
            ones_sb = cp.tile([1, 128], F32, tag="ones")
            nc.sync.dma_start(ones_sb[:], ones_in.ap())

            es_sb = {}
            ax_sb = {}
            sx_sb = {}
            idx_sb = {}
            for S in (0, 1):
                es_sb[S] = cp.tile([128, tot[S]], F32, tag=f"es{S}", name=f"es_sb{S}")
                nc.sync.dma_start(es_sb[S][:], es_in[S].ap())
                ax_sb[S] = cp.tile([128, padr // 16], I16, tag=f"ax{S}", name=f"ax_sb{S}")
                nc.sync.dma_start(ax_sb[S][:], ax_in[S].ap())
                sx_sb[S] = cp.tile([128, padr // 16], I16, tag=f"sx{S}", name=f"sx_sb{S}")
                nc.sync.dma_start(sx_sb[S][:], sx_in[S].ap())
                for l in (0, 1):
                    idx_sb[(l, S)] = cp.tile([128, tot[S] * 8], I16, tag=f"idx{l}{S}", name=f"idx_sb{l}{S}")
                    nc.sync.dma_start(idx_sb[(l, S)][:], idx_in[(l, S)].ap())

            rawsc = cp.tile([128, nwin, hidden], F32, tag="rawsc")
            a_cur = cp.tile([128, nwin, 2], F32, tag="a_cur")
            ostash = cp.tile([128, nwin, out_ch], F32, tag="ostash")
            s_all = cp.tile([128, nwin], F32, tag="s_all")
            ls_all = cp.tile([128, nwin], F32, tag="ls_all")
            zz = cp.tile([128, 128], F32, tag="zz")
            nc.vector.memset(zz[:], 0.0)

            # ------------- table pass: ext0 rows = [h0 bf16 | b0 f32] -------------
            CB = 8  # chunks per batched DMA
            with tc.tile_pool(name="prep", bufs=3) as pp, \
                 tc.tile_pool(name="prep_ps", bufs=4, space="PSUM") as pps:
                for cg in range(0, nchunk, CB):
                    nb = min(CB, nchunk - cg)
                    xt_sb = pp.tile([128, CB * kt, 128], BF16, tag="xt")
                    nc.sync.dma_start(
                        xt_sb[:, 0:nb * kt, :],
                        xt.ap()[cg:cg + nb, :, :].rearrange(
                            "c (k p) r -> p (c k) r", p=128))
                    xb_sb = pp.tile([1, CB, 128], BF16, tag="xb")
                    nc.sync.dma_start(xb_sb[:, 0:nb, :], xball.ap()[cg:cg + nb, :])
                    ext_sb = pp.tile([128, CB, hh + 1], F32, tag="ext")
                    for ci in range(nb):
                        ps = pps.tile([128, hidden], F32, tag="h0ps")
                        for k in range(kt):
                            nc.tensor.matmul(ps[:], xt_sb[:, ci * kt + k, :], t1wt_sb[:, k, :],
                                             start=(k == 0), stop=False)
                        nc.tensor.matmul(ps[:], xb_sb[:, ci, :], t1b_sb[:], start=False, stop=True)
                        hbf = ext_sb[:, ci, 0:hh].bitcast(BF16)
                        nc.scalar.activation(hbf, ps[:], mybir.ActivationFunctionType.Relu)
                        scr = pp.tile([128, hidden], BF16, tag="scr")
                        _mr(nc, scr[:], hbf, gw_sb[:, 1, :], ext_sb[:, ci, hh:hh + 1])
                    nc.sync.dma_start(
                        bass.AP(ext0, cg * 128 * 128,
                                [[128, 128], [128 * 128, nb], [1, hh + 1]]),
                        ext_sb[:, 0:nb, :])

                # own-row passes: v=0 natural (rawsc), v=1/2 per-structure (a0)
                for v in range(3):
                    for wg in range(0, nwin, CB):
                        nb = min(CB, nwin - wg)
                        xt_sb = pp.tile([128, CB * kt, 128], BF16, tag="xt")
                        nc.sync.dma_start(
                            xt_sb[:, 0:nb * kt, :],
                            xto.ap()[v, wg:wg + nb, :, :].rearrange(
                                "c (k p) r -> p (c k) r", p=128))
                        xb_sb = pp.tile([1, CB, 128], BF16, tag="xb")
                        nc.sync.dma_start(xb_sb[:, 0:nb, :], xtob.ap()[v, wg:wg + nb, :])
                        for ci in range(nb):
                            w = wg + ci
                            ps = pps.tile([128, hidden], F32, tag="h0ps")
                            for k in range(kt):
                                nc.tensor.matmul(ps[:], xt_sb[:, ci * kt + k, :], t1wt_sb[:, k, :],
                                                 start=(k == 0), stop=False)
                            nc.tensor.matmul(ps[:], xb_sb[:, ci, :], t1b_sb[:], start=False, stop=True)
                            if v == 0:
                                nc.vector.tensor_scalar(out=rawsc[:, w, :], in0=ps[:],
                                                        scalar1=0.0, scalar2=eps,
                                                        op0=mybir.AluOpType.max,
                                                        op1=mybir.AluOpType.mult)
                            else:
                                hb = pp.tile([128, hidden], BF16, tag="hb")
                                nc.scalar.activation(hb[:], ps[:], mybir.ActivationFunctionType.Relu)
                                scr = pp.tile([128, hidden], BF16, tag="scr")
                                _mr(nc, scr[:], hb[:], gw_sb[:, 0, :],
                                    a_cur[:, w, v - 1:v])

            nc.vector.tensor_scalar(
                out=a_cur[:].rearrange("p w s -> p (w s)"), in0=a_cur[:].rearrange("p w s -> p (w s)"),
                scalar1=gb_sb[:, 0:1], scalar2=None, op0=mybir.AluOpType.add)

            # ------------- edge phase -------------
            def emit_layer(l, table, lo_base, hi_base):
                # zero the accumulator
                with tc.tile_pool(name=f"zr{l}", bufs=1) as zp:
                    for w in range(nwin):
                        nc.sync.dma_start(agg.ap()[w * 128:(w + 1) * 128, :], zz[:])
                if os.environ.get("KSKIP_EDGE"):
                    return
                if l == 1:
                    # a1 per structure: gather own agi rows in perm order + ttr
                    with tc.tile_pool(name=f"ag{l}", bufs=2) as ap_, \
                         tc.tile_pool(name=f"ags{l}", bufs=2) as asp:
                        for S in (0, 1):
                            GA = ap_.tile([128, nwin, 128], F32, tag="GA")
                            for wg in range(0, nwin, 16):
                                nw_g = min(16, nwin - wg)
                                nc.gpsimd.dma_gather(
                                    out_ap=GA[:, wg:wg + nw_g, :], in_ap=agi.ap(),
                                    idxs_ap=ax_sb[S][:, wg * 8:(wg + nw_g) * 8],
                                    num_idxs=nw_g * 128, num_idxs_reg=nw_g * 128,
                                    elem_size=128,
                                    single_packet=False, queue_num=wg % knq)
                            for w in range(nwin):
                                scr = asp.tile([128, hidden], BF16, tag="scr")
                                _mr(nc, scr[:], GA[:, w, 0:hh].bitcast(BF16),
                                    gw_sb[:, 2, :], a_cur[:, w, S:S + 1])
                        nc.vector.tensor_scalar(
                            out=a_cur[:].rearrange("p w s -> p (w s)"),
                            in0=a_cur[:].rearrange("p w s -> p (w s)"),
                            scalar1=gb_sb[:, 1:2], scalar2=None,
                            op0=mybir.AluOpType.add)

                with tc.tile_pool(name=f"g{l}", bufs=3) as gp, \
                     tc.tile_pool(name=f"m{l}", bufs=3) as mp, \
                     tc.tile_pool(name=f"s{l}", bufs=4) as sp, \
                     tc.tile_pool(name=f"st{l}", bufs=2) as stp, \
                     tc.tile_pool(name=f"psW{l}", bufs=2, space="PSUM") as psW:
                    qi = 0
                    for S in (0, 1):
                        W_ps = None
                        stage = None
                        for (c0, ns, w, first, last) in pieces[S]:
                            if first and w % GW == 0:
                                stage = stp.tile([128, GW, 128], F32, tag="stage")
                            if first:
                                W_ps = psW.tile([128, hidden], F32, tag="W")
                            nt = ns * 128
                            if os.environ.get("KPROBE256"):
                                G = gp.tile([128, MAXSLOT, 64], F32, tag="G")
                                nc.gpsimd.dma_gather(
                                    out_ap=G[:, 0:ns, :],
                                    in_ap=(table.ap()[hi_base:, 0:64] if S == 1 else table.ap()[lo_base:, 0:64]),
                                    idxs_ap=idx_sb[(l, S)][:, c0 * 8:(c0 + ns) * 8],
                                    num_idxs=nt, num_idxs_reg=nt, elem_size=64,
                                    elem_step=128,
                                    single_packet=False, queue_num=qi % knq)
                            else:
                                G = gp.tile([128, MAXSLOT, 128], F32, tag="G")
                                nc.gpsimd.dma_gather(
                                    out_ap=G[:, 0:ns, :],
                                    in_ap=(table.ap()[hi_base:, :] if S == 1 else table.ap()[lo_base:, :]),
                                    idxs_ap=idx_sb[(l, S)][:, c0 * 8:(c0 + ns) * 8],
                                    num_idxs=nt, num_idxs_reg=nt, elem_size=128,
                                    single_packet=(os.environ.get("KGSP") == "1"),
                                    queue_num=qi % knq)
                            qi += 1
                            tk = sp.tile([128, MAXSLOT], F32, tag="tk")
                            bcol = hh - 1 if os.environ.get("KPROBE256") else hh
                            nc.scalar.activation(
                                tk[:, 0:ns], G[:, 0:ns, bcol:bcol + 1].squeeze(2),
                                mybir.ActivationFunctionType.Tanh,
                                bias=a_cur[:, w, S:S + 1])
                            wtb = sp.tile([128, MAXSLOT],
                                          F32 if os.environ.get("KDIS_MSGB") else BF16,
                                          tag="wtb")
                            nc.vector.tensor_tensor(
                                out=wtb[:, 0:ns], in0=tk[:, 0:ns],
                                in1=es_sb[S][:, c0:c0 + ns],
                                op=mybir.AluOpType.mult)
                            msg = mp.tile([128, MAXSLOT, 128], BF16, tag="msg")
                            if os.environ.get("KDIS_MSGB"):
                                for s in range(ns):
                                    nc.vector.tensor_scalar(
                                        out=msg[:, s, :],
                                        in0=G[:, s, 0:hh].bitcast(BF16),
                                        scalar1=wtb[:, s:s + 1], scalar2=None,
                                        op0=mybir.AluOpType.mult)
                            else:
                                nc.vector.tensor_tensor(
                                    out=msg[:, 0:ns, :],
                                    in0=G[:, 0:ns, 0:hh].bitcast(BF16),
                                    in1=wtb[:, 0:ns].unsqueeze(2).broadcast_to((128, ns, 128)),
                                    op=mybir.AluOpType.mult)
                            if os.environ.get("KMM1"):
                                for s in range(ns):
                                    nc.tensor.matmul(W_ps[:], identb_sb[:], msg[:, s, :],
                                                     start=(first and s == 0),
                                                     stop=(last and s == ns - 1))
                            else:
                                MSL = 4
                                for s in range(0, ns, MSL):
                                    nsl = min(MSL, ns - s)
                                    nc.tensor.matmul(
                                        W_ps[:].unsqueeze(1).broadcast_to((128, nsl, 128)),
                                        identb_sb[:],
                                        msg[:, s:s + nsl, :],
                                        start=(first and s == 0),
                                        stop=(last and s + nsl == ns))
                            if last:
                                j = w % GW
                                if os.environ.get("KDIS_COPY"):
                                    nc.vector.tensor_copy(stage[:, j, :], W_ps[:])
                                else:
                                    nc.scalar.activation(stage[:, j, :], W_ps[:],
                                                         mybir.ActivationFunctionType.Copy)
                                if w % GW == GW - 1 or w == nwin - 1:
                                    gn = j + 1
                                    if os.environ.get("KDIS_SCAT"):
                                        nc.sync.dma_start(
                                            agg.ap()[(w - j) * 128:(w + 1) * 128, :],
                                            stage[:, 0:gn, :])
                                    else:
                                        nc.gpsimd.dma_scatter_add(
                                            out_ap=agg.ap(),
                                            in_ap=stage[:, 0:gn, :],
                                            idxs_ap=sx_sb[S][:, (w - j) * 8:(w + 1) * 8],
                                            num_idxs=gn * 128, num_idxs_reg=gn * 128,
                                            elem_size=128,
                                            single_packet=(os.environ.get("KSCAT_SP") == "1"),
                                            queue_num=qi % knq)
                                    qi += 1

                # ------------- combine -------------
                with tc.tile_pool(name=f"cb{l}", bufs=4) as cb, \
                     tc.tile_pool(name=f"cps{l}", bufs=2, space="PSUM") as cps:
                    for w in range(nwin):
                        rows = 128 if w < nwin - 1 else last_win_rows
                        A_sb = cb.tile([128, 128], F32, tag="A")
                        nc.sync.dma_start(A_sb[:], agg.ap()[w * 128:(w + 1) * 128, :])
                        h_sb = cb.tile([128, hidden], F32, tag="h")
                        nc.vector.tensor_tensor(out=h_sb[:], in0=A_sb[:],
                                                in1=rawsc[:, w, :],
                                                op=mybir.AluOpType.add)
                        if l == 0:
                            ext1 = cb.tile([128, hh + 1], F32, tag="ext1")
                            h1b = ext1[:, 0:hh].bitcast(BF16)
                            if os.environ.get("KDIS_COPY"):
                                nc.vector.tensor_copy(h1b, h_sb[:])
                            else:
                                nc.scalar.activation(h1b, h_sb[:], mybir.ActivationFunctionType.Copy)
                            scr3 = cb.tile([128, hidden], BF16, tag="scr3")
                            _mr(nc, scr3[:], h1b, gw_sb[:, 3, :], ext1[:, hh:hh + 1])
                            nc.sync.dma_start(agi.ap()[w * 128:(w + 1) * 128, 0:hh + 1], ext1[:])
                        else:
                            ht_ps = cps.tile([128, 128], F32, tag="ht")
                            nc.tensor.matmul(ht_ps[:], h_sb[:], identf_sb[:],
                                             start=True, stop=True)
                            ht_sb = cb.tile([128, 128], F32, tag="ht_sb")
                            if os.environ.get("KDIS_COPY"):
                                nc.vector.tensor_copy(ht_sb[:], ht_ps[:])
                            else:
                                nc.scalar.activation(ht_sb[:], ht_ps[:], mybir.ActivationFunctionType.Copy)
                            o_ps = cps.tile([128, out_ch], F32, tag="ops")
                            nc.tensor.matmul(o_ps[:], ht_sb[:], t2wt_sb[:],
                                             start=True, stop=False)
                            nc.tensor.matmul(o_ps[:], ones_sb[:], t2b_sb[:],
                                             start=False, stop=True)
                            nm = cb.tile([128, 1], F32, tag="nm")
                            nc.vector.reduce_max(out=nm[:], in_=o_ps[:],
                                                 axis=mybir.AxisListType.X,
                                                 negate=True)
                            e_sb = cb.tile([128, out_ch], F32, tag="e")
                            if os.environ.get("KDIS_ACC"):
                                nc.scalar.activation(e_sb[:], o_ps[:],
                                                     mybir.ActivationFunctionType.Exp,
                                                     bias=nm[:])
                                nc.vector.reduce_sum(out=s_all[:, w:w + 1], in_=e_sb[:],
                                                     axis=mybir.AxisListType.X)
                            else:
                                nc.scalar.activation(e_sb[:], o_ps[:],
                                                     mybir.ActivationFunctionType.Exp,
                                                     bias=nm[:],
                                                     accum_out=s_all[:, w:w + 1])
                            nc.vector.tensor_scalar(out=ostash[:, w, :], in0=o_ps[:],
                                                    scalar1=nm[:], scalar2=None,
                                                    op0=mybir.AluOpType.add)

            emit_layer(0, ext0, 0, lo_split)
            if os.environ.get("KSKIP_CC"):
                for c in range(ncores):
                    nc.sync.dma_start(ago.ap()[c * padr:(c + 1) * padr, :], agi.ap())
            else:
                if not os.environ.get("KCCSTRIDED"):
                    nc.gpsimd.collective_compute(
                        "AllGather", mybir.AluOpType.bypass,
                        replica_groups=[list(range(ncores))],
                        ins=[agi.ap().opt()], outs=[ago.ap().opt()])
                else:
                    nc.gpsimd.collective_compute(
                        "AllGather", mybir.AluOpType.bypass,
                        replica_groups=[list(range(ncores))],
                        ins=[agi.ap()[:, 0:hh + 1].opt()],
                        outs=[ago.ap()[:, 0:hh + 1].opt()])
            if not os.environ.get("KSKIP_L1"):
                emit_layer(1, ago, 0, hibase)

            # ------------- head epilogue -------------
            if os.environ.get("KSKIP_L1"):
                with tc.tile_pool(name="ep0", bufs=1) as ep0:
                    oz = ep0.tile([128, out_ch], F32, tag="oz")
                    nc.vector.memset(oz[:], 0.0)
                    for w in range(nwin):
                        rows = 128 if w < nwin - 1 else last_win_rows
                        nc.sync.dma_start(out.ap()[w * 128:w * 128 + rows, :], oz[0:rows, :])
                return nc
            with tc.tile_pool(name="ep", bufs=4) as ep:
                nc.scalar.activation(ls_all[:], s_all[:],
                                     mybir.ActivationFunctionType.Ln)
                for w in range(nwin):
                    rows = 128 if w < nwin - 1 else last_win_rows
                    o_sb = ep.tile([128, out_ch], F32, tag="o")
                    nc.vector.tensor_scalar(out=o_sb[:], in0=ostash[:, w, :],
                                            scalar1=ls_all[:, w:w + 1], scalar2=None,
                                            op0=mybir.AluOpType.subtract)
                    nc.sync.dma_start(out.ap()[w * 128:w * 128 + rows, :],
                                      o_sb[0:rows, :])

    return nc


# ======================================================================
# Host driver
# ======================================================================

def _bf16(a):
    import ml_dtypes
    return np.asarray(a, dtype=ml_dtypes.bfloat16)


def kernel_run(x, edge_index, t1_w, t1_b, gate_w, gate_b, t2_w, t2_b,
               n_nodes=N_NODES, in_ch=IN_CH, hidden=HIDDEN, out_ch=OUT_CH,
               eps=EPS, ncores=NCORES, lo_split=None, trace=False):
    _install_profile_hook()
    from concourse import bass_utils

    r_per = n_nodes // ncores
    if lo_split is None:
        lo_split = (ncores // 2) * r_per
    meta = preprocess(edge_index, n_nodes, ncores, lo_split)
    nwin = meta["nwin"]
    nchunk = (n_nodes + 127) // 128

    nc = build_kernel(meta, n_nodes, in_ch, hidden, out_ch, eps, lo_split, ncores)
    nc.finalize()

    # host arrays
    x = np.asarray(x, np.float32)
    xT = np.concatenate([x.T, np.ones((1, x.shape[0]), np.float32)], axis=0)  # [in+1, N]
    pad_n = nchunk * 128
    xT_pad = np.zeros((in_ch + 1, pad_n), np.float32)
    xT_pad[:, :n_nodes] = xT
    xt_tiled = _bf16(np.ascontiguousarray(
        xT_pad[:in_ch].reshape(in_ch, nchunk, 128).transpose(1, 0, 2)))
    xball_h = _bf16(np.ascontiguousarray(xT_pad[in_ch].reshape(nchunk, 128)))
    t1wt_h = _bf16(np.concatenate([np.asarray(t1_w, np.float32).T,
                                   np.asarray(t1_b, np.float32)[None, :]], axis=0))
    gw = np.asarray(gate_w, np.float32)
    gwrep_h = _bf16(np.stack([
        np.tile(gw[0, :hidden][None, :], (128, 1)),
        np.tile(gw[0, hidden:][None, :], (128, 1)),
        np.tile(gw[1, :hidden][None, :], (128, 1)),
        np.tile(gw[1, hidden:][None, :], (128, 1))]))
    gbrep_h = np.tile(np.asarray(gate_b, np.float32)[None, :], (128, 1))
    t2wt_h = np.ascontiguousarray(np.asarray(t2_w, np.float32).T)
    t2b_h = np.asarray(t2_b, np.float32)[None, :]
    identf_h = np.eye(128, dtype=np.float32)
    identb_h = _bf16(np.eye(128, dtype=np.float32))
    ones_h = np.ones((1, 128), np.float32)

    pad_own = nwin * 128
    in_maps = []
    for c in range(ncores):
        sd = meta["percore"][c]
        # own-row x copies: natural + per-structure perms
        xto_h = np.zeros((3, nwin, in_ch, 128), np.float32)
        xtob_h = np.zeros((3, nwin, 128), np.float32)
        for v in range(3):
            if v == 0:
                sl = np.zeros((in_ch + 1, pad_own), np.float32)
                take = min(pad_own, n_nodes - c * r_per)
                sl[:, :take] = xT[:, c * r_per: c * r_per + take]
            else:
                perm = sd[v - 1]["perm"]
                gl = c * r_per + perm  # [r_per]
                sl = np.zeros((in_ch + 1, pad_own), np.float32)
                sl[:, :r_per] = xT[:, gl]
            xto_h[v] = sl[:in_ch].reshape(in_ch, nwin, 128).transpose(1, 0, 2)
            xtob_h[v] = sl[in_ch].reshape(nwin, 128)
        im = {
            "xt": xt_tiled, "xball": xball_h, "xto": _bf16(xto_h),
            "xtob": _bf16(xtob_h), "t1wt": t1wt_h,
            "gwrep": gwrep_h, "gbrep": gbrep_h, "t2wt": t2wt_h, "t2b": t2b_h,
            "identf": identf_h, "identb": identb_h, "ones": ones_h,
        }
        for S in (0, 1):
            im[f"idx0{S}"] = sd[S]["idx0"]
            im[f"idx1{S}"] = sd[S]["idx1"]
            im[f"es{S}"] = sd[S]["es"]
            im[f"ax{S}"] = sd[S]["aidx"]
            im[f"sx{S}"] = sd[S]["sidx"]
        in_maps.append(im)

    res = bass_utils.run_bass_kernel_spmd(
        nc, in_maps, core_ids=list(range(ncores)), trace=trace)
    outp = np.concatenate([res.results[c]["out"] for c in range(ncores)], axis=0)
    return outp[:n_nodes], res


def kernel(**inputs):
    x = inputs["x"]
    edge_index = inputs["edge_index"]
    outp, _ = kernel_run(
        x, edge_index, inputs["t1_w"], inputs["t1_b"], inputs["gate_w"],
        inputs["gate_b"], inputs["t2_w"], inputs["t2_b"])
    return np.asarray(outp, np.float32)
